# revision 26
# baseline (speedup 1.0000x reference)
"""GAT (2-layer) Trainium2 Bass kernel — 8-core SPMD, pipelined gathers.

Sharding: dst nodes across 8 cores (12500 each). Per core, dsts are packed
into 98 windows of 128 (one SBUF partition per dst), profile-sorted (lexsort
on per-src-group degree vectors) so slot padding is small. Edges are gathered
per window as rows [a_s f32 | h bf16] from per-layer node tables
(4 src-groups, int16 indices) via tile-managed gpsimd.dma_gather — no
critical sections, 4 SWDGE queues and a 3-deep staging pool, so gathers
pipeline with DVE compute automatically. Slot layout per batch is
[group][window][Lmax_g] (per-group uniform Lmax), which makes every
attention op a single strided DVE instruction and segment-sums a
two-stage tensor_reduce.
Pad slots point at a row with a_s=-300 => weights ~e^-56, no masking needed.
Epilogues (ELU + layer-2 projection, log_softmax) run batched over all
windows at layer end; layer-2 node table is built per-shard and AllGathered.
"""

import numpy as np
import ml_dtypes

import concourse.bacc as bacc
import concourse.bass as bass
import concourse.mybir as mybir
import concourse.tile as tile
from concourse.bass_utils import run_bass_kernel_spmd
from concourse.masks import make_identity

F32 = mybir.dt.float32
BF16 = mybir.dt.bfloat16
I16 = mybir.dt.int16
AX = mybir.AxisListType
OP = mybir.AluOpType
ACT = mybir.ActivationFunctionType

N, E = 100000, 1600000
IN, HID, OUT, HEADS = 256, 16, 64, 8
NEG = 0.2
NCORES = 8
NSH = N // NCORES        # 12500
NGRP = 4
GSZ = N // NGRP          # 25000
NP = 25088               # padded rows per group (196*128)
NW = 98                  # windows per core
SH_ROWS = NW * 128       # 12544
PAD1 = GSZ               # group-local pad row, table1 (25000; rows 25000.. zero-x)
PAD2 = NSH               # pair-local pad row, table2 (row 12500 of even shard)
MAXC_BUDGET = 88         # slot columns per gather batch
WPB = 1                  # windows per batch
ROW1 = 256               # bf16 elems per table1 row (512B)
ROW2 = 128               # bf16 elems per table2 row (256B)
A_S_NEG = -300.0
TPB = 196                # 128-node tiles per src group
CHUNK = 14               # tiles per phase-0 staging chunk


# ---------------------------------------------------------------- host side
def _layout(src, dst):
    core = dst // NSH
    grp = src // GSZ
    cg_all = np.zeros((NCORES, NSH, NGRP), np.int32)
    np.add.at(cg_all, (core, dst % NSH, grp), 1)
    perms = []
    for k in range(NCORES):
        cg = cg_all[k]
        perms.append(np.lexsort((cg[:, 3], cg[:, 2], cg[:, 1], cg[:, 0]))[::-1])
    Lg = np.zeros((NW, NGRP), np.int64)
    for k in range(NCORES):
        cgp = cg_all[k][perms[k]]
        cgp = np.concatenate([cgp, np.zeros((SH_ROWS - NSH, NGRP), np.int32)])
        Lg = np.maximum(Lg, cgp.reshape(NW, 128, NGRP).max(axis=1))
    Lmaxw = Lg.max(axis=1)           # per-window uniform slot count
    sig = np.empty(N, np.int64)
    for k in range(NCORES):
        pos = np.empty(NSH, np.int64)
        pos[perms[k]] = np.arange(NSH)
        sig[k * NSH:(k + 1) * NSH] = k * SH_ROWS + pos
    eorder = np.lexsort((grp, dst))
    es, ed, eg, ec = src[eorder], dst[eorder], grp[eorder], core[eorder]
    core_starts = np.searchsorted(ec, np.arange(NCORES + 1))
    cores = [(es[a:b], (ed[a:b] - k * NSH), eg[a:b])
             for k, (a, b) in enumerate(zip(core_starts[:-1], core_starts[1:]))]
    return dict(Lg=Lg, Lmaxw=Lmaxw, perms=perms, sig=sig, cores=cores)


def _make_batches(Lg):
    """Batches of <=WPB windows; per-group uniform Lmax within a batch.
    Returns [(ws, lmg)] with lmg = per-group slot count (len NGRP)."""
    batches = []
    w = 0
    while w < NW:
        take = 1
        if (w + 1 < NW) and WPB >= 2:
            lmg = Lg[w:w + 2].max(axis=0)
            if 2 * int(lmg.sum()) <= MAXC_BUDGET:
                take = 2
        ws = list(range(w, w + take))
        lmg = [max(1, int(v)) for v in Lg[ws].max(axis=0)]
        batches.append((ws, lmg))
        w += take
    return batches


def _pack_idx(arr_pj):
    """[128, cols] slot-array of indices -> wrapped idx tile [128, cols*8]."""
    I = arr_pj.T.ravel()                      # I[j*128+p]
    W = I.reshape(-1, 16).T.astype(np.int16)  # [16, len/16]
    return np.tile(W, (8, 1))


def _host_inputs(inputs, lay, batches):
    x = np.asarray(inputs["x"], np.float32)
    W1 = np.asarray(inputs["W1"], np.float64)
    att1_s = np.asarray(inputs["att1_s"], np.float64)
    att1_d = np.asarray(inputs["att1_d"], np.float64)
    W2 = np.asarray(inputs["W2"], np.float64)
    att2_s = np.asarray(inputs["att2_s"], np.float64)
    att2_d = np.asarray(inputs["att2_d"], np.float64)
    b1 = np.asarray(inputs["b1"], np.float32)
    b2 = np.asarray(inputs["b2"], np.float32)
    Lg, perms, sig = lay["Lg"], lay["perms"], lay["sig"]

    A_s = np.zeros((HEADS * HID, HEADS))
    A_d = np.zeros((HEADS * HID, HEADS))
    for h in range(HEADS):
        A_s[h * HID:(h + 1) * HID, h] = att1_s[h]
        A_d[h * HID:(h + 1) * HID, h] = att1_d[h]
    w1r = np.concatenate([W1, W1 @ A_s, W1 @ A_d], axis=1)          # [256,144]
    w2r = np.concatenate([W2, W2 @ att2_s.T, W2 @ att2_d.T], axis=1)  # [128,66]
    w1r_bf = w1r.astype(ml_dtypes.bfloat16)
    w2r_bf = w2r.astype(ml_dtypes.bfloat16)

    xT = np.zeros((IN, NGRP * NP), np.float32)
    for g in range(NGRP):
        xT[:, g * NP:g * NP + GSZ] = x[g * GSZ:(g + 1) * GSZ].T
    xT_bf = xT.astype(ml_dtypes.bfloat16)

    common = {
        "xt0": np.ascontiguousarray(xT_bf[:128]),
        "xt1": np.ascontiguousarray(xT_bf[128:]),
        "w1r0": np.ascontiguousarray(w1r_bf[:128]),
        "w1r1": np.ascontiguousarray(w1r_bf[128:]),
        "w2r": np.ascontiguousarray(w2r_bf),
        "b1rep": np.ascontiguousarray(np.tile(b1[None, :], (128, 1)).astype(np.float32)),
        "b2rep": np.ascontiguousarray(np.tile(b2[None, :], (128, 1)).astype(np.float32)),
    }

    per_core = []
    for k in range(NCORES):
        es, edl, eg = lay["cores"][k]
        pos = np.empty(NSH, np.int64)
        pos[perms[k]] = np.arange(NSH)
        o = np.lexsort((eg, pos[edl]))
        es_o, eg_o, pos_o = es[o], eg[o], pos[edl][o]
        w_o, p_o = pos_o // 128, pos_o % 128
        key = pos_o * NGRP + eg_o
        slot = np.arange(len(o)) - np.searchsorted(key, key)
        idx1_secs, idx2_secs = [], []
        for ws, lmg in batches:
            Wn = len(ws)
            for g in range(NGRP):
                lm = lmg[g]
                a1 = np.full((128, Wn * lm), PAD1, np.int64)
                a2 = np.full((128, Wn * lm), PAD2, np.int64)
                for wl, w in enumerate(ws):
                    m = (w_o == w) & (eg_o == g)
                    pp, jj, ss = p_o[m], slot[m], es_o[m]
                    a1[pp, wl * lm + jj] = ss % GSZ
                    a2[pp, wl * lm + jj] = sig[ss] % NP
                idx1_secs.append(a1)
                idx2_secs.append(a2)
        idx1 = np.concatenate([_pack_idx(a) for a in idx1_secs], axis=1)
        idx2 = np.concatenate([_pack_idx(a) for a in idx2_secs], axis=1)
        xtp = np.zeros((IN, SH_ROWS), np.float32)
        xtp[:, :NSH] = x[k * NSH:(k + 1) * NSH].T[:, perms[k]]
        xtp_bf = xtp.astype(ml_dtypes.bfloat16)
        d = dict(common)
        d["idx1"] = np.ascontiguousarray(idx1)
        d["idx2"] = np.ascontiguousarray(idx2)
        d["xtp0"] = np.ascontiguousarray(xtp_bf[:128])
        d["xtp1"] = np.ascontiguousarray(xtp_bf[128:])
        per_core.append(d)
    return per_core


# ------------------------------------------------------------- device side
GATHER_VARIANT = "full"   # "full" | "tiny" (timing experiment: 128-row gathers)


def _build_program(batches, nrepeat=1):
    nc = bacc.Bacc("TRN2", target_bir_lowering=False, debug=False,
                   num_devices=NCORES, num_swdge_queues=4)
    MAXC = max(len(ws) * sum(lmg) for ws, lmg in batches)
    IDXF = sum(len(ws) * sum(lmg) * 8 for ws, lmg in batches)
    xt0 = nc.declare_dram_parameter("xt0", [128, NGRP * NP], BF16, isOutput=False)
    xt1 = nc.declare_dram_parameter("xt1", [128, NGRP * NP], BF16, isOutput=False)
    w1r0 = nc.declare_dram_parameter("w1r0", [128, 144], BF16, isOutput=False)
    w1r1 = nc.declare_dram_parameter("w1r1", [128, 144], BF16, isOutput=False)
    w2rp = nc.declare_dram_parameter("w2r", [128, 66], BF16, isOutput=False)
    b1rep = nc.declare_dram_parameter("b1rep", [128, 128], F32, isOutput=False)
    b2rep = nc.declare_dram_parameter("b2rep", [128, 64], F32, isOutput=False)
    idx1 = nc.declare_dram_parameter("idx1", [128, IDXF], I16, isOutput=False)
    idx2 = nc.declare_dram_parameter("idx2", [128, IDXF], I16, isOutput=False)
    xtp0 = nc.declare_dram_parameter("xtp0", [128, SH_ROWS], BF16, isOutput=False)
    xtp1 = nc.declare_dram_parameter("xtp1", [128, SH_ROWS], BF16, isOutput=False)
    outp = nc.declare_dram_parameter("out", [SH_ROWS, OUT], F32, isOutput=True)

    table1 = nc.dram_tensor("table1", [NGRP * NP, ROW1], BF16)
    shard2 = nc.dram_tensor("shard2", [SH_ROWS, ROW2], BF16)
    table2 = nc.dram_tensor("table2", [NCORES * SH_ROWS, ROW2], BF16,
                            addr_space="Shared")

    with tile.TileContext(nc) as tc:
        with tc.tile_pool(name="const", bufs=1) as constp:
            w1r0_t = constp.tile([128, 144], BF16, tag="w1r0")
            w1r1_t = constp.tile([128, 144], BF16, tag="w1r1")
            w2r_t = constp.tile([128, 66], BF16, tag="w2r")
            b1_t = constp.tile([128, 128], F32, tag="b1")
            b2_t = constp.tile([128, 64], F32, tag="b2")
            ident = constp.tile([128, 128], BF16, tag="ident")
            adwin = constp.tile([128, NW * HEADS], F32, tag="adwin")
            ad2win = constp.tile([128, NW], F32, tag="ad2win")
            padrow = constp.tile([128, ROW1], BF16, tag="padrow")
            pr2 = constp.tile([1, 2], BF16, tag="pr2")
            nc.sync.dma_start(out=w1r0_t[:], in_=w1r0[:])
            nc.sync.dma_start(out=w1r1_t[:], in_=w1r1[:])
            nc.sync.dma_start(out=w2r_t[:], in_=w2rp[:])
            nc.sync.dma_start(out=b1_t[:], in_=b1rep[:])
            nc.sync.dma_start(out=b2_t[:], in_=b2rep[:])
            make_identity(nc, ident[:])
            nc.vector.memset(padrow[:], 0.0)
            nc.vector.memset(padrow[0:1, 0:16].bitcast(F32), A_S_NEG)
            nc.vector.memset(pr2[0:1, 0:2].bitcast(F32), A_S_NEG)

            for _rep in range(nrepeat):
                _emit_body(nc, tc, batches, MAXC,
                           dict(xt0=xt0, xt1=xt1, idx1=idx1, idx2=idx2,
                                xtp0=xtp0, xtp1=xtp1, outp=outp,
                                table1=table1, shard2=shard2, table2=table2,
                                w1r0_t=w1r0_t, w1r1_t=w1r1_t, w2r_t=w2r_t,
                                b1_t=b1_t, b2_t=b2_t, ident=ident,
                                adwin=adwin, ad2win=ad2win, padrow=padrow,
                                pr2=pr2))
    nc.compile()
    return nc


def _emit_body(nc, tc, batches, MAXC, T):
    xt0, xt1 = T["xt0"], T["xt1"]
    table1, shard2, table2 = T["table1"], T["shard2"], T["table2"]
    w1r0_t, w1r1_t, w2r_t = T["w1r0_t"], T["w1r1_t"], T["w2r_t"]
    adwin, ad2win = T["adwin"], T["ad2win"]

    # ---------------- phase 0: dense h1 table (all nodes) ----------
    PSG = 3  # psum group: 3 node-tiles per PSUM bank
    with (
        tc.tile_pool(name="xt", bufs=2) as xtpool,
        tc.tile_pool(name="psum0", bufs=4, space="PSUM") as psump,
        tc.tile_pool(name="rows", bufs=2) as rowsp,
    ):
        for g in range(NGRP):
            for blk in range(TPB // CHUNK):
                base = g * NP + blk * CHUNK * 128
                xs0 = xtpool.tile([128, CHUNK * 128], BF16, tag="xs0")
                xs1 = xtpool.tile([128, CHUNK * 128], BF16, tag="xs1")
                nc.sync.dma_start(out=xs0[:], in_=xt0[:, base:base + CHUNK * 128])
                nc.sync.dma_start(out=xs1[:], in_=xt1[:, base:base + CHUNK * 128])
                rows = rowsp.tile([128, CHUNK * ROW1], BF16, tag="rows")
                t = 0
                while t < CHUNK:
                    npg = min(PSG, CHUNK - t)
                    ps = psump.tile([128, PSG * 144], F32, tag="ps0")
                    for u in range(npg):
                        tt = t + u
                        nc.tensor.matmul(
                            out=ps[:, u * 144:(u + 1) * 144],
                            lhsT=xs0[:, tt * 128:(tt + 1) * 128],
                            rhs=w1r0_t[:], start=True, stop=False)
                        nc.tensor.matmul(
                            out=ps[:, u * 144:(u + 1) * 144],
                            lhsT=xs1[:, tt * 128:(tt + 1) * 128],
                            rhs=w1r1_t[:], start=False, stop=True)
                    # a_s: rows[:, t*256 + {0..16}] (strided over npg tiles)
                    rv = rows[:, t * ROW1:(t + npg) * ROW1]
                    nc.vector.tensor_copy(
                        out=rv.rearrange("p (a r) -> p a r", a=npg)[:, :, 0:16]
                            .bitcast(F32),
                        in_=ps[:, 0:npg * 144].rearrange("p (a r) -> p a r", a=npg)
                            [:, :, 128:136])
                    nc.vector.tensor_copy(
                        out=rv.rearrange("p (a r) -> p a r", a=npg)[:, :, 16:144],
                        in_=ps[:, 0:npg * 144].rearrange("p (a r) -> p a r", a=npg)
                            [:, :, 0:128])
                    t += npg
                nc.sync.dma_start(
                    out=table1[base:base + CHUNK * 128, :]
                        .rearrange("(a p) r -> p a r", p=128),
                    in_=rows[:].rearrange("p (a r) -> p a r", a=CHUNK))
        # pad row: a_s := -300 (h stays 0) on group-local row PAD1
        for g in range(NGRP):
            nc.sync.dma_start(out=table1[g * NP + PAD1:g * NP + PAD1 + 1, :],
                              in_=T["padrow"][0:1, :])

        # a_d per window (window-ordered x.T): 14 windows per chunk
        for blk in range(NW // CHUNK):
            base = blk * CHUNK * 128
            xp0 = xtpool.tile([128, CHUNK * 128], BF16, tag="xp0")
            xp1 = xtpool.tile([128, CHUNK * 128], BF16, tag="xp1")
            nc.sync.dma_start(out=xp0[:], in_=T["xtp0"][:, base:base + CHUNK * 128])
            nc.sync.dma_start(out=xp1[:], in_=T["xtp1"][:, base:base + CHUNK * 128])
            half = CHUNK // 2  # 7 windows per psum tile
            for hb in range(2):
                psa = psump.tile([128, 7 * 16], F32, tag="psa")
                for u in range(half):
                    wloc = hb * half + u
                    nc.tensor.matmul(out=psa[:, u * 16:(u + 1) * 16],
                                     lhsT=xp0[:, wloc * 128:(wloc + 1) * 128],
                                     rhs=w1r0_t[:, 128:144], start=True, stop=False)
                    nc.tensor.matmul(out=psa[:, u * 16:(u + 1) * 16],
                                     lhsT=xp1[:, wloc * 128:(wloc + 1) * 128],
                                     rhs=w1r1_t[:, 128:144], start=False, stop=True)
                w0 = blk * CHUNK + hb * half
                nc.vector.tensor_copy(
                    out=adwin[:, w0 * 8:(w0 + half) * 8]
                        .rearrange("p (a h) -> p a h", a=half),
                    in_=psa[:].rearrange("p (a r) -> p a r", a=half)[:, :, 8:16])

    # ---------------- edge layers ----------------------------------
    def edge_layer(layer, den_t, opre_t):
        tabl, row_e = (table1, ROW1) if layer == 1 else (table2, ROW2)
        idxin = T["idx1"] if layer == 1 else T["idx2"]
        nh = HEADS if layer == 1 else 1
        nch = HID if layer == 1 else OUT
        hcw = nh * nch                      # 128 / 64
        hoff = 16 if layer == 1 else 2      # bf16 slots before h
        adv_t = adwin if layer == 1 else ad2win
        idx_off = 0

        with (
            tc.tile_pool(name=f"stag{layer}", bufs=4) as stagp,
            tc.tile_pool(name=f"idx{layer}", bufs=2) as idxp,
            tc.tile_pool(name=f"wall{layer}", bufs=2) as wallp,
        ):
            for ws, lmg in batches:
                Wn = len(ws)
                assert Wn == 1, "edge_layer assumes single-window batches"
                lsum = sum(lmg)
                cols = Wn * lsum
                goff = [Wn * sum(lmg[:g]) for g in range(NGRP + 1)]
                ixt = idxp.tile([128, MAXC * 8], I16, tag="ix")
                nc.sync.dma_start(
                    out=ixt[:, 0:cols * 8],
                    in_=idxin[:, idx_off:idx_off + cols * 8])
                idx_off += cols * 8
                stag = stagp.tile([128, MAXC * ROW1], BF16, tag="st")
                for g in range(NGRP):
                    nidx_g = 128 * Wn * lmg[g]
                    ncols_g = Wn * lmg[g]
                    if GATHER_VARIANT == "tiny":
                        nidx_g, ncols_g = 128, 1
                    sl3 = stag[:, goff[g] * row_e:(goff[g] + ncols_g) * row_e] \
                        .rearrange("p (k d) -> p k d", d=row_e)
                    nc.gpsimd.dma_gather(
                        out_ap=sl3, in_ap=tabl[g * NP:(g + 1) * NP, :],
                        idxs_ap=ixt[:, goff[g] * 8:goff[g] * 8 + nidx_g // 16],
                        num_idxs=nidx_g, num_idxs_reg=nidx_g,
                        elem_size=row_e, single_packet=False,
                        queue_num=g)
                # single window per batch: the 4 group sections are
                # contiguous and share one a_d, so every op is whole-batch
                w = ws[0]
                sec = stag[:, 0:cols * row_e]
                wall_t = wallp.tile([128, MAXC * HEADS], F32, tag="wa")
                lr_t = wallp.tile([128, MAXC * HEADS], F32, tag="lr")
                wall = wall_t[:, 0:cols * nh]
                a_s = sec.rearrange("p (l d) -> p l d", d=row_e) \
                    [:, :, 0:2 * nh].bitcast(F32)
                adv = adv_t[:, w * nh:(w + 1) * nh] \
                    .rearrange("p (a h) -> p a h", a=1) \
                    .to_broadcast([128, cols, nh])
                nc.vector.tensor_tensor(
                    out=wall.rearrange("p (l h) -> p l h", h=nh),
                    in0=a_s, in1=adv, op=OP.add)
                # leaky-relu + exp on the whole batch
                nc.vector.tensor_scalar_mul(lr_t[:, 0:cols * nh], wall, NEG)
                nc.vector.tensor_tensor(out=wall, in0=wall,
                                        in1=lr_t[:, 0:cols * nh], op=OP.max)
                nc.scalar.activation(wall, wall, ACT.Exp, 0.0, 1.0)
                # weighted messages, in place over the gathered h
                hview = sec.rearrange("p (x d) -> p x d", d=row_e) \
                    [:, :, hoff:hoff + hcw] \
                    .rearrange("p x (h c) -> p x h c", h=nh)
                wb = wall.rearrange("p (x h c) -> p x h c", h=nh, c=1) \
                    .to_broadcast([128, cols, nh, nch])
                meng = nc.vector if (w % 2 == 0) else nc.gpsimd
                meng.tensor_tensor(out=hview, in0=hview, in1=wb, op=OP.mult)
                # one-stage segment reduction over all slots of the window
                nc.vector.tensor_reduce(
                    out=opre_t[:, w * hcw:(w + 1) * hcw],
                    in_=sec.rearrange("p (x d) -> p d x", d=row_e)
                        [:, hoff:hoff + hcw, :],
                    axis=AX.X, op=OP.add)
                nc.vector.tensor_reduce(
                    out=den_t[:, w * nh:(w + 1) * nh],
                    in_=wall.rearrange("p (x h) -> p h x", h=nh),
                    axis=AX.X, op=OP.add)

    with tc.tile_pool(name="acc1", bufs=1) as acc1p:
        den_all = acc1p.tile([128, NW * HEADS], F32, tag="den1")
        opre_all = acc1p.tile([128, NW * 128], F32, tag="opre1")
        edge_layer(1, den_all, opre_all)

        # ------------- epilogue 1: normalize, ELU, project, shard2 ----
        with (
            tc.tile_pool(name="epi1", bufs=1) as epip,
            tc.tile_pool(name="sm1", bufs=4) as smallp,
            tc.tile_pool(name="psum1", bufs=4, space="PSUM") as psump,
        ):
        nc.vector.tensor_scalar_max(den_all[:], den_all[:], 1e-30)
        rec = epip.tile([128, NW * HEADS], F32, tag="rec")
        nc.vector.reciprocal(rec[:], den_all[:])
        o1 = opre_all
        nc.vector.tensor_tensor(
            out=o1[:].rearrange("p (v c) -> p v c", c=HID),
            in0=o1[:].rearrange("p (v c) -> p v c", c=HID),
            in1=rec[:].rearrange("p (v c) -> p v c", c=1)
                .to_broadcast([128, NW * HEADS, HID]),
            op=OP.mult)
        nc.vector.tensor_tensor(
            out=o1[:].rearrange("p (w x) -> p w x", x=128),
            in0=o1[:].rearrange("p (w x) -> p w x", x=128),
            in1=T["b1_t"][:].rearrange("p (a x) -> p a x", a=1)
                .to_broadcast([128, NW, 128]),
            op=OP.add)
        tneg = epip.tile([128, NW * 128], F32, tag="tneg")
        nc.vector.tensor_scalar_min(tneg[:], o1[:], 0.0)
        nc.scalar.activation(tneg[:], tneg[:], ACT.Exp, 0.0, 1.0)
        nc.vector.tensor_relu(o1[:], o1[:])
        nc.vector.tensor_tensor(out=o1[:], in0=o1[:], in1=tneg[:], op=OP.add)
        nc.vector.tensor_scalar_add(o1[:], o1[:], -1.0)
        o1bf = epip.tile([128, NW * 128], BF16, tag="o1bf")
        nc.vector.tensor_copy(out=o1bf[:], in_=o1[:])
        row2_all = epip.tile([128, NW * ROW2], BF16, tag="row2")
        nc.vector.memset(row2_all[:], 0.0)
        for w in range(NW):
            pst = psump.tile([128, 128], BF16, tag="pst")
            nc.tensor.transpose(out=pst[:], in_=o1bf[:, w * 128:(w + 1) * 128],
                                identity=T["ident"][:])
            o1T = smallp.tile([128, 128], BF16, tag="o1T")
            nc.vector.tensor_copy(out=o1T[:], in_=pst[:])
            ps2 = psump.tile([128, 66], F32, tag="ps2")
            nc.tensor.matmul(out=ps2[:], lhsT=o1T[:], rhs=w2r_t[:],
                             start=True, stop=True)
            nc.vector.tensor_copy(
                out=row2_all[:, w * ROW2:w * ROW2 + 2].bitcast(F32),
                in_=ps2[:, 64:65])
            nc.vector.tensor_copy(out=row2_all[:, w * ROW2 + 2:w * ROW2 + 66],
                                  in_=ps2[:, 0:64])
            nc.vector.tensor_copy(out=ad2win[:, w:w + 1], in_=ps2[:, 65:66])
        nc.sync.dma_start(out=shard2[:].rearrange("(w p) r -> p w r", p=128),
                          in_=row2_all[:].rearrange("p (w r) -> p w r", r=ROW2))
        nc.sync.dma_start(out=shard2[PAD2:PAD2 + 1, 0:2], in_=T["pr2"][0:1, :])

    nc.gpsimd.collective_compute(
        "AllGather", OP.bypass,
        replica_groups=[list(range(NCORES))],
        ins=[shard2[:]], outs=[table2[:]],
    )

    with tc.tile_pool(name="acc2", bufs=1) as acc2p:
        den2 = acc2p.tile([128, NW], F32, tag="den2")
        opre2 = acc2p.tile([128, NW * OUT], F32, tag="opre2")
        edge_layer(2, den2, opre2)

        # ------------- epilogue 2: normalize, bias, log_softmax -------
        _epilogue2(nc, tc, T, den2, opre2)


def _epilogue2(nc, tc, T, den2, opre2):
    with tc.tile_pool(name="epi2", bufs=1) as epip:
        nc.vector.tensor_scalar_max(den2[:], den2[:], 1e-30)
        rec2 = epip.tile([128, NW], F32, tag="rec2")
        nc.vector.reciprocal(rec2[:], den2[:])
        o2 = opre2
        nc.vector.tensor_tensor(
            out=o2[:].rearrange("p (w c) -> p w c", c=OUT),
            in0=o2[:].rearrange("p (w c) -> p w c", c=OUT),
            in1=rec2[:].rearrange("p (w c) -> p w c", c=1)
                .to_broadcast([128, NW, OUT]),
            op=OP.mult)
        nc.vector.tensor_tensor(
            out=o2[:].rearrange("p (w x) -> p w x", x=OUT),
            in0=o2[:].rearrange("p (w x) -> p w x", x=OUT),
            in1=T["b2_t"][:].rearrange("p (a x) -> p a x", a=1)
                .to_broadcast([128, NW, OUT]),
            op=OP.add)
        mx = epip.tile([128, NW], F32, tag="mx")
        nc.vector.tensor_reduce(
            out=mx[:], in_=o2[:].rearrange("p (w c) -> p w c", c=OUT),
            axis=AX.X, op=OP.max)
        nc.vector.tensor_tensor(
            out=o2[:].rearrange("p (w c) -> p w c", c=OUT),
            in0=o2[:].rearrange("p (w c) -> p w c", c=OUT),
            in1=mx[:].rearrange("p (w c) -> p w c", c=1)
                .to_broadcast([128, NW, OUT]),
            op=OP.subtract)
        ex = epip.tile([128, NW * OUT], F32, tag="ex")
        nc.scalar.activation(ex[:], o2[:], ACT.Exp, 0.0, 1.0)
        se = epip.tile([128, NW], F32, tag="se")
        nc.vector.tensor_reduce(
            out=se[:], in_=ex[:].rearrange("p (w c) -> p w c", c=OUT),
            axis=AX.X, op=OP.add)
        ln = epip.tile([128, NW], F32, tag="ln")
        nc.scalar.activation(ln[:], se[:], ACT.Ln, 0.0, 1.0)
        nc.vector.tensor_tensor(
            out=ex[:].rearrange("p (w c) -> p w c", c=OUT),
            in0=o2[:].rearrange("p (w c) -> p w c", c=OUT),
            in1=ln[:].rearrange("p (w c) -> p w c", c=1)
                .to_broadcast([128, NW, OUT]),
            op=OP.subtract)
        nc.sync.dma_start(out=T["outp"][:].rearrange("(w p) f -> p w f", p=128),
                          in_=ex[:].rearrange("p (w f) -> p w f", f=OUT))


_CACHE = {}


def kernel(**inputs):
    ei = np.asarray(inputs["edge_index"])
    src, dst = ei[0].astype(np.int64), ei[1].astype(np.int64)
    lay = _layout(src, dst)
    batches = _make_batches(lay["Lg"])
    per_core = _host_inputs(inputs, lay, batches)
    key = (ei.tobytes()[:64], tuple(tuple(lmg) for _, lmg in batches))
    if key not in _CACHE:
        _CACHE[key] = _build_program(batches)
    nc = _CACHE[key]
    res = run_bass_kernel_spmd(nc, per_core, core_ids=list(range(NCORES)))
    out = np.empty((N, OUT), np.float32)
    for k in range(NCORES):
        out[k * NSH + lay["perms"][k]] = res.results[k]["out"][:NSH]
    return out


if __name__ == "__main__":
    d = np.load("/root/problem/_inp_check.npz")
    o = kernel(**{k: d[k] for k in d.files})
    ref = np.load("/root/problem/_ref_check.npy")
    rel = np.linalg.norm(o - ref) / np.linalg.norm(ref)
    err = np.abs(o - ref) / (np.abs(ref) + 1e-5)
    print("fro rel err:", rel, "max elem rel err:", err.max())


# revision 27
# speedup vs baseline: 1.3637x; 1.3637x over previous
"""GAT (2-layer) Trainium2 Bass kernel — 8-core SPMD, pipelined gathers.

Sharding: dst nodes across 8 cores (12500 each). Per core, dsts are packed
into 98 windows of 128 (one SBUF partition per dst), profile-sorted (lexsort
on per-src-group degree vectors) so slot padding is small. Edges are gathered
per window as rows [a_s f32 | h bf16] from per-layer node tables
(4 src-groups, int16 indices) via tile-managed gpsimd.dma_gather — no
critical sections, 4 SWDGE queues and a 3-deep staging pool, so gathers
pipeline with DVE compute automatically. Slot layout per batch is
[group][window][Lmax_g] (per-group uniform Lmax), which makes every
attention op a single strided DVE instruction and segment-sums a
two-stage tensor_reduce.
Pad slots point at a row with a_s=-300 => weights ~e^-56, no masking needed.
Epilogues (ELU + layer-2 projection, log_softmax) run batched over all
windows at layer end; layer-2 node table is built per-shard and AllGathered.
"""

import numpy as np
import ml_dtypes

import concourse.bacc as bacc
import concourse.bass as bass
import concourse.mybir as mybir
import concourse.tile as tile
from concourse.bass_utils import run_bass_kernel_spmd
from concourse.masks import make_identity

F32 = mybir.dt.float32
BF16 = mybir.dt.bfloat16
I16 = mybir.dt.int16
AX = mybir.AxisListType
OP = mybir.AluOpType
ACT = mybir.ActivationFunctionType

N, E = 100000, 1600000
IN, HID, OUT, HEADS = 256, 16, 64, 8
NEG = 0.2
NCORES = 8
NSH = N // NCORES        # 12500
NGRP = 4
GSZ = N // NGRP          # 25000
NP = 25088               # padded rows per group (196*128)
NW = 98                  # windows per core
SH_ROWS = NW * 128       # 12544
PAD1 = GSZ               # group-local pad row, table1 (25000; rows 25000.. zero-x)
PAD2 = NSH               # pair-local pad row, table2 (row 12500 of even shard)
MAXC_BUDGET = 88         # slot columns per gather batch
WPB = 1                  # windows per batch
ROW1 = 256               # bf16 elems per table1 row (512B)
ROW2 = 128               # bf16 elems per table2 row (256B)
A_S_NEG = -300.0
TPB = 196                # 128-node tiles per src group
CHUNK = 14               # tiles per phase-0 staging chunk


# ---------------------------------------------------------------- host side
def _layout(src, dst):
    core = dst // NSH
    grp = src // GSZ
    cg_all = np.zeros((NCORES, NSH, NGRP), np.int32)
    np.add.at(cg_all, (core, dst % NSH, grp), 1)
    perms = []
    for k in range(NCORES):
        cg = cg_all[k]
        perms.append(np.lexsort((cg[:, 3], cg[:, 2], cg[:, 1], cg[:, 0]))[::-1])
    Lg = np.zeros((NW, NGRP), np.int64)
    for k in range(NCORES):
        cgp = cg_all[k][perms[k]]
        cgp = np.concatenate([cgp, np.zeros((SH_ROWS - NSH, NGRP), np.int32)])
        Lg = np.maximum(Lg, cgp.reshape(NW, 128, NGRP).max(axis=1))
    Lmaxw = Lg.max(axis=1)           # per-window uniform slot count
    sig = np.empty(N, np.int64)
    for k in range(NCORES):
        pos = np.empty(NSH, np.int64)
        pos[perms[k]] = np.arange(NSH)
        sig[k * NSH:(k + 1) * NSH] = k * SH_ROWS + pos
    eorder = np.lexsort((grp, dst))
    es, ed, eg, ec = src[eorder], dst[eorder], grp[eorder], core[eorder]
    core_starts = np.searchsorted(ec, np.arange(NCORES + 1))
    cores = [(es[a:b], (ed[a:b] - k * NSH), eg[a:b])
             for k, (a, b) in enumerate(zip(core_starts[:-1], core_starts[1:]))]
    return dict(Lg=Lg, Lmaxw=Lmaxw, perms=perms, sig=sig, cores=cores)


def _make_batches(Lg):
    """Batches of <=WPB windows; per-group uniform Lmax within a batch.
    Returns [(ws, lmg)] with lmg = per-group slot count (len NGRP)."""
    batches = []
    w = 0
    while w < NW:
        take = 1
        if (w + 1 < NW) and WPB >= 2:
            lmg = Lg[w:w + 2].max(axis=0)
            if 2 * int(lmg.sum()) <= MAXC_BUDGET:
                take = 2
        ws = list(range(w, w + take))
        lmg = [max(1, int(v)) for v in Lg[ws].max(axis=0)]
        batches.append((ws, lmg))
        w += take
    return batches


def _pack_idx(arr_pj):
    """[128, cols] slot-array of indices -> wrapped idx tile [128, cols*8]."""
    I = arr_pj.T.ravel()                      # I[j*128+p]
    W = I.reshape(-1, 16).T.astype(np.int16)  # [16, len/16]
    return np.tile(W, (8, 1))


def _host_inputs(inputs, lay, batches):
    x = np.asarray(inputs["x"], np.float32)
    W1 = np.asarray(inputs["W1"], np.float64)
    att1_s = np.asarray(inputs["att1_s"], np.float64)
    att1_d = np.asarray(inputs["att1_d"], np.float64)
    W2 = np.asarray(inputs["W2"], np.float64)
    att2_s = np.asarray(inputs["att2_s"], np.float64)
    att2_d = np.asarray(inputs["att2_d"], np.float64)
    b1 = np.asarray(inputs["b1"], np.float32)
    b2 = np.asarray(inputs["b2"], np.float32)
    Lg, perms, sig = lay["Lg"], lay["perms"], lay["sig"]

    A_s = np.zeros((HEADS * HID, HEADS))
    A_d = np.zeros((HEADS * HID, HEADS))
    for h in range(HEADS):
        A_s[h * HID:(h + 1) * HID, h] = att1_s[h]
        A_d[h * HID:(h + 1) * HID, h] = att1_d[h]
    w1r = np.concatenate([W1, W1 @ A_s, W1 @ A_d], axis=1)          # [256,144]
    w2r = np.concatenate([W2, W2 @ att2_s.T, W2 @ att2_d.T], axis=1)  # [128,66]
    w1r_bf = w1r.astype(ml_dtypes.bfloat16)
    w2r_bf = w2r.astype(ml_dtypes.bfloat16)

    xT = np.zeros((IN, NGRP * NP), np.float32)
    for g in range(NGRP):
        xT[:, g * NP:g * NP + GSZ] = x[g * GSZ:(g + 1) * GSZ].T
    xT_bf = xT.astype(ml_dtypes.bfloat16)

    common = {
        "xt0": np.ascontiguousarray(xT_bf[:128]),
        "xt1": np.ascontiguousarray(xT_bf[128:]),
        "w1r0": np.ascontiguousarray(w1r_bf[:128]),
        "w1r1": np.ascontiguousarray(w1r_bf[128:]),
        "w2r": np.ascontiguousarray(w2r_bf),
        "b1rep": np.ascontiguousarray(np.tile(b1[None, :], (128, 1)).astype(np.float32)),
        "b2rep": np.ascontiguousarray(np.tile(b2[None, :], (128, 1)).astype(np.float32)),
    }

    per_core = []
    for k in range(NCORES):
        es, edl, eg = lay["cores"][k]
        pos = np.empty(NSH, np.int64)
        pos[perms[k]] = np.arange(NSH)
        o = np.lexsort((eg, pos[edl]))
        es_o, eg_o, pos_o = es[o], eg[o], pos[edl][o]
        w_o, p_o = pos_o // 128, pos_o % 128
        key = pos_o * NGRP + eg_o
        slot = np.arange(len(o)) - np.searchsorted(key, key)
        idx1_secs, idx2_secs = [], []
        for ws, lmg in batches:
            Wn = len(ws)
            for g in range(NGRP):
                lm = lmg[g]
                a1 = np.full((128, Wn * lm), PAD1, np.int64)
                a2 = np.full((128, Wn * lm), PAD2, np.int64)
                for wl, w in enumerate(ws):
                    m = (w_o == w) & (eg_o == g)
                    pp, jj, ss = p_o[m], slot[m], es_o[m]
                    a1[pp, wl * lm + jj] = ss % GSZ
                    a2[pp, wl * lm + jj] = sig[ss] % NP
                idx1_secs.append(a1)
                idx2_secs.append(a2)
        idx1 = np.concatenate([_pack_idx(a) for a in idx1_secs], axis=1)
        idx2 = np.concatenate([_pack_idx(a) for a in idx2_secs], axis=1)
        xtp = np.zeros((IN, SH_ROWS), np.float32)
        xtp[:, :NSH] = x[k * NSH:(k + 1) * NSH].T[:, perms[k]]
        xtp_bf = xtp.astype(ml_dtypes.bfloat16)
        d = dict(common)
        d["idx1"] = np.ascontiguousarray(idx1)
        d["idx2"] = np.ascontiguousarray(idx2)
        d["xtp0"] = np.ascontiguousarray(xtp_bf[:128])
        d["xtp1"] = np.ascontiguousarray(xtp_bf[128:])
        per_core.append(d)
    return per_core


# ------------------------------------------------------------- device side
GATHER_VARIANT = "full"   # "full" | "tiny" (timing experiment: 128-row gathers)


def _build_program(batches, nrepeat=1):
    nc = bacc.Bacc("TRN2", target_bir_lowering=False, debug=False,
                   num_devices=NCORES, num_swdge_queues=4)
    MAXC = max(len(ws) * sum(lmg) for ws, lmg in batches)
    IDXF = sum(len(ws) * sum(lmg) * 8 for ws, lmg in batches)
    xt0 = nc.declare_dram_parameter("xt0", [128, NGRP * NP], BF16, isOutput=False)
    xt1 = nc.declare_dram_parameter("xt1", [128, NGRP * NP], BF16, isOutput=False)
    w1r0 = nc.declare_dram_parameter("w1r0", [128, 144], BF16, isOutput=False)
    w1r1 = nc.declare_dram_parameter("w1r1", [128, 144], BF16, isOutput=False)
    w2rp = nc.declare_dram_parameter("w2r", [128, 66], BF16, isOutput=False)
    b1rep = nc.declare_dram_parameter("b1rep", [128, 128], F32, isOutput=False)
    b2rep = nc.declare_dram_parameter("b2rep", [128, 64], F32, isOutput=False)
    idx1 = nc.declare_dram_parameter("idx1", [128, IDXF], I16, isOutput=False)
    idx2 = nc.declare_dram_parameter("idx2", [128, IDXF], I16, isOutput=False)
    xtp0 = nc.declare_dram_parameter("xtp0", [128, SH_ROWS], BF16, isOutput=False)
    xtp1 = nc.declare_dram_parameter("xtp1", [128, SH_ROWS], BF16, isOutput=False)
    outp = nc.declare_dram_parameter("out", [SH_ROWS, OUT], F32, isOutput=True)

    table1 = nc.dram_tensor("table1", [NGRP * NP, ROW1], BF16)
    shard2 = nc.dram_tensor("shard2", [SH_ROWS, ROW2], BF16)
    table2 = nc.dram_tensor("table2", [NCORES * SH_ROWS, ROW2], BF16,
                            addr_space="Shared")

    with tile.TileContext(nc) as tc:
        with tc.tile_pool(name="const", bufs=1) as constp:
            w1r0_t = constp.tile([128, 144], BF16, tag="w1r0")
            w1r1_t = constp.tile([128, 144], BF16, tag="w1r1")
            w2r_t = constp.tile([128, 66], BF16, tag="w2r")
            b1_t = constp.tile([128, 128], F32, tag="b1")
            b2_t = constp.tile([128, 64], F32, tag="b2")
            ident = constp.tile([128, 128], BF16, tag="ident")
            adwin = constp.tile([128, NW * HEADS], F32, tag="adwin")
            ad2win = constp.tile([128, NW], F32, tag="ad2win")
            padrow = constp.tile([128, ROW1], BF16, tag="padrow")
            pr2 = constp.tile([1, 2], BF16, tag="pr2")
            nc.sync.dma_start(out=w1r0_t[:], in_=w1r0[:])
            nc.sync.dma_start(out=w1r1_t[:], in_=w1r1[:])
            nc.sync.dma_start(out=w2r_t[:], in_=w2rp[:])
            nc.sync.dma_start(out=b1_t[:], in_=b1rep[:])
            nc.sync.dma_start(out=b2_t[:], in_=b2rep[:])
            make_identity(nc, ident[:])
            nc.vector.memset(padrow[:], 0.0)
            nc.vector.memset(padrow[0:1, 0:16].bitcast(F32), A_S_NEG)
            nc.vector.memset(pr2[0:1, 0:2].bitcast(F32), A_S_NEG)

            for _rep in range(nrepeat):
                _emit_body(nc, tc, batches, MAXC,
                           dict(xt0=xt0, xt1=xt1, idx1=idx1, idx2=idx2,
                                xtp0=xtp0, xtp1=xtp1, outp=outp,
                                table1=table1, shard2=shard2, table2=table2,
                                w1r0_t=w1r0_t, w1r1_t=w1r1_t, w2r_t=w2r_t,
                                b1_t=b1_t, b2_t=b2_t, ident=ident,
                                adwin=adwin, ad2win=ad2win, padrow=padrow,
                                pr2=pr2))
    nc.compile()
    return nc


def _emit_body(nc, tc, batches, MAXC, T):
    xt0, xt1 = T["xt0"], T["xt1"]
    table1, shard2, table2 = T["table1"], T["shard2"], T["table2"]
    w1r0_t, w1r1_t, w2r_t = T["w1r0_t"], T["w1r1_t"], T["w2r_t"]
    adwin, ad2win = T["adwin"], T["ad2win"]

    # ---------------- phase 0: dense h1 table (all nodes) ----------
    PSG = 3  # psum group: 3 node-tiles per PSUM bank
    with (
        tc.tile_pool(name="xt", bufs=2) as xtpool,
        tc.tile_pool(name="psum0", bufs=4, space="PSUM") as psump,
        tc.tile_pool(name="rows", bufs=2) as rowsp,
    ):
        for g in range(NGRP):
            for blk in range(TPB // CHUNK):
                base = g * NP + blk * CHUNK * 128
                xs0 = xtpool.tile([128, CHUNK * 128], BF16, tag="xs0")
                xs1 = xtpool.tile([128, CHUNK * 128], BF16, tag="xs1")
                nc.sync.dma_start(out=xs0[:], in_=xt0[:, base:base + CHUNK * 128])
                nc.sync.dma_start(out=xs1[:], in_=xt1[:, base:base + CHUNK * 128])
                rows = rowsp.tile([128, CHUNK * ROW1], BF16, tag="rows")
                t = 0
                while t < CHUNK:
                    npg = min(PSG, CHUNK - t)
                    ps = psump.tile([128, PSG * 144], F32, tag="ps0")
                    for u in range(npg):
                        tt = t + u
                        nc.tensor.matmul(
                            out=ps[:, u * 144:(u + 1) * 144],
                            lhsT=xs0[:, tt * 128:(tt + 1) * 128],
                            rhs=w1r0_t[:], start=True, stop=False)
                        nc.tensor.matmul(
                            out=ps[:, u * 144:(u + 1) * 144],
                            lhsT=xs1[:, tt * 128:(tt + 1) * 128],
                            rhs=w1r1_t[:], start=False, stop=True)
                    # a_s: rows[:, t*256 + {0..16}] (strided over npg tiles)
                    rv = rows[:, t * ROW1:(t + npg) * ROW1]
                    nc.vector.tensor_copy(
                        out=rv.rearrange("p (a r) -> p a r", a=npg)[:, :, 0:16]
                            .bitcast(F32),
                        in_=ps[:, 0:npg * 144].rearrange("p (a r) -> p a r", a=npg)
                            [:, :, 128:136])
                    nc.vector.tensor_copy(
                        out=rv.rearrange("p (a r) -> p a r", a=npg)[:, :, 16:144],
                        in_=ps[:, 0:npg * 144].rearrange("p (a r) -> p a r", a=npg)
                            [:, :, 0:128])
                    t += npg
                nc.sync.dma_start(
                    out=table1[base:base + CHUNK * 128, :]
                        .rearrange("(a p) r -> p a r", p=128),
                    in_=rows[:].rearrange("p (a r) -> p a r", a=CHUNK))
        # pad row: a_s := -300 (h stays 0) on group-local row PAD1
        for g in range(NGRP):
            nc.sync.dma_start(out=table1[g * NP + PAD1:g * NP + PAD1 + 1, :],
                              in_=T["padrow"][0:1, :])

        # a_d per window (window-ordered x.T): 14 windows per chunk
        for blk in range(NW // CHUNK):
            base = blk * CHUNK * 128
            xp0 = xtpool.tile([128, CHUNK * 128], BF16, tag="xp0")
            xp1 = xtpool.tile([128, CHUNK * 128], BF16, tag="xp1")
            nc.sync.dma_start(out=xp0[:], in_=T["xtp0"][:, base:base + CHUNK * 128])
            nc.sync.dma_start(out=xp1[:], in_=T["xtp1"][:, base:base + CHUNK * 128])
            half = CHUNK // 2  # 7 windows per psum tile
            for hb in range(2):
                psa = psump.tile([128, 7 * 16], F32, tag="psa")
                for u in range(half):
                    wloc = hb * half + u
                    nc.tensor.matmul(out=psa[:, u * 16:(u + 1) * 16],
                                     lhsT=xp0[:, wloc * 128:(wloc + 1) * 128],
                                     rhs=w1r0_t[:, 128:144], start=True, stop=False)
                    nc.tensor.matmul(out=psa[:, u * 16:(u + 1) * 16],
                                     lhsT=xp1[:, wloc * 128:(wloc + 1) * 128],
                                     rhs=w1r1_t[:, 128:144], start=False, stop=True)
                w0 = blk * CHUNK + hb * half
                nc.vector.tensor_copy(
                    out=adwin[:, w0 * 8:(w0 + half) * 8]
                        .rearrange("p (a h) -> p a h", a=half),
                    in_=psa[:].rearrange("p (a r) -> p a r", a=half)[:, :, 8:16])

    # ---------------- edge layers ----------------------------------
    def edge_layer(layer, den_t, opre_t):
        tabl, row_e = (table1, ROW1) if layer == 1 else (table2, ROW2)
        idxin = T["idx1"] if layer == 1 else T["idx2"]
        nh = HEADS if layer == 1 else 1
        nch = HID if layer == 1 else OUT
        hcw = nh * nch                      # 128 / 64
        hoff = 16 if layer == 1 else 2      # bf16 slots before h
        adv_t = adwin if layer == 1 else ad2win
        idx_off = 0

        with (
            tc.tile_pool(name=f"stag{layer}", bufs=4) as stagp,
            tc.tile_pool(name=f"idx{layer}", bufs=2) as idxp,
            tc.tile_pool(name=f"wall{layer}", bufs=2) as wallp,
        ):
            for ws, lmg in batches:
                Wn = len(ws)
                assert Wn == 1, "edge_layer assumes single-window batches"
                lsum = sum(lmg)
                cols = Wn * lsum
                goff = [Wn * sum(lmg[:g]) for g in range(NGRP + 1)]
                ixt = idxp.tile([128, MAXC * 8], I16, tag="ix")
                nc.sync.dma_start(
                    out=ixt[:, 0:cols * 8],
                    in_=idxin[:, idx_off:idx_off + cols * 8])
                idx_off += cols * 8
                stag = stagp.tile([128, MAXC * ROW1], BF16, tag="st")
                for g in range(NGRP):
                    nidx_g = 128 * Wn * lmg[g]
                    ncols_g = Wn * lmg[g]
                    if GATHER_VARIANT == "tiny":
                        nidx_g, ncols_g = 128, 1
                    sl3 = stag[:, goff[g] * row_e:(goff[g] + ncols_g) * row_e] \
                        .rearrange("p (k d) -> p k d", d=row_e)
                    nc.gpsimd.dma_gather(
                        out_ap=sl3, in_ap=tabl[g * NP:(g + 1) * NP, :],
                        idxs_ap=ixt[:, goff[g] * 8:goff[g] * 8 + nidx_g // 16],
                        num_idxs=nidx_g, num_idxs_reg=nidx_g,
                        elem_size=row_e, single_packet=False,
                        queue_num=g)
                # single window per batch: the 4 group sections are
                # contiguous and share one a_d, so every op is whole-batch
                w = ws[0]
                sec = stag[:, 0:cols * row_e]
                wall_t = wallp.tile([128, MAXC * HEADS], F32, tag="wa")
                lr_t = wallp.tile([128, MAXC * HEADS], F32, tag="lr")
                wall = wall_t[:, 0:cols * nh]
                a_s = sec.rearrange("p (l d) -> p l d", d=row_e) \
                    [:, :, 0:2 * nh].bitcast(F32)
                adv = adv_t[:, w * nh:(w + 1) * nh] \
                    .rearrange("p (a h) -> p a h", a=1) \
                    .to_broadcast([128, cols, nh])
                nc.vector.tensor_tensor(
                    out=wall.rearrange("p (l h) -> p l h", h=nh),
                    in0=a_s, in1=adv, op=OP.add)
                # leaky-relu + exp on the whole batch
                nc.vector.tensor_scalar_mul(lr_t[:, 0:cols * nh], wall, NEG)
                nc.vector.tensor_tensor(out=wall, in0=wall,
                                        in1=lr_t[:, 0:cols * nh], op=OP.max)
                nc.scalar.activation(wall, wall, ACT.Exp, 0.0, 1.0)
                # weighted messages, in place over the gathered h
                hview = sec.rearrange("p (x d) -> p x d", d=row_e) \
                    [:, :, hoff:hoff + hcw] \
                    .rearrange("p x (h c) -> p x h c", h=nh)
                wb = wall.rearrange("p (x h c) -> p x h c", h=nh, c=1) \
                    .to_broadcast([128, cols, nh, nch])
                nc.vector.tensor_tensor(out=hview, in0=hview, in1=wb, op=OP.mult)
                # one-stage segment reduction over all slots of the window
                nc.vector.tensor_reduce(
                    out=opre_t[:, w * hcw:(w + 1) * hcw],
                    in_=sec.rearrange("p (x d) -> p d x", d=row_e)
                        [:, hoff:hoff + hcw, :],
                    axis=AX.X, op=OP.add)
                nc.vector.tensor_reduce(
                    out=den_t[:, w * nh:(w + 1) * nh],
                    in_=wall.rearrange("p (x h) -> p h x", h=nh),
                    axis=AX.X, op=OP.add)

    with tc.tile_pool(name="acc1", bufs=1) as acc1p:
        den_all = acc1p.tile([128, NW * HEADS], F32, tag="den1")
        opre_all = acc1p.tile([128, NW * 128], F32, tag="opre1")
        edge_layer(1, den_all, opre_all)

        # ------------- epilogue 1: normalize, ELU, project, shard2 ----
        with (
            tc.tile_pool(name="epi1", bufs=1) as epip,
            tc.tile_pool(name="sm1", bufs=4) as smallp,
            tc.tile_pool(name="psum1", bufs=4, space="PSUM") as psump,
        ):
        nc.vector.tensor_scalar_max(den_all[:], den_all[:], 1e-30)
        rec = epip.tile([128, NW * HEADS], F32, tag="rec")
        nc.vector.reciprocal(rec[:], den_all[:])
        o1 = opre_all
        nc.vector.tensor_tensor(
            out=o1[:].rearrange("p (v c) -> p v c", c=HID),
            in0=o1[:].rearrange("p (v c) -> p v c", c=HID),
            in1=rec[:].rearrange("p (v c) -> p v c", c=1)
                .to_broadcast([128, NW * HEADS, HID]),
            op=OP.mult)
        nc.vector.tensor_tensor(
            out=o1[:].rearrange("p (w x) -> p w x", x=128),
            in0=o1[:].rearrange("p (w x) -> p w x", x=128),
            in1=T["b1_t"][:].rearrange("p (a x) -> p a x", a=1)
                .to_broadcast([128, NW, 128]),
            op=OP.add)
        tneg = epip.tile([128, NW * 128], F32, tag="tneg")
        nc.vector.tensor_scalar_min(tneg[:], o1[:], 0.0)
        nc.scalar.activation(tneg[:], tneg[:], ACT.Exp, 0.0, 1.0)
        nc.vector.tensor_relu(o1[:], o1[:])
        nc.vector.tensor_tensor(out=o1[:], in0=o1[:], in1=tneg[:], op=OP.add)
        nc.vector.tensor_scalar_add(o1[:], o1[:], -1.0)
        o1bf = epip.tile([128, NW * 128], BF16, tag="o1bf")
        nc.vector.tensor_copy(out=o1bf[:], in_=o1[:])
        row2_all = epip.tile([128, NW * ROW2], BF16, tag="row2")
        nc.vector.memset(row2_all[:], 0.0)
        for w in range(NW):
            pst = psump.tile([128, 128], BF16, tag="pst")
            nc.tensor.transpose(out=pst[:], in_=o1bf[:, w * 128:(w + 1) * 128],
                                identity=T["ident"][:])
            o1T = smallp.tile([128, 128], BF16, tag="o1T")
            nc.vector.tensor_copy(out=o1T[:], in_=pst[:])
            ps2 = psump.tile([128, 66], F32, tag="ps2")
            nc.tensor.matmul(out=ps2[:], lhsT=o1T[:], rhs=w2r_t[:],
                             start=True, stop=True)
            nc.vector.tensor_copy(
                out=row2_all[:, w * ROW2:w * ROW2 + 2].bitcast(F32),
                in_=ps2[:, 64:65])
            nc.vector.tensor_copy(out=row2_all[:, w * ROW2 + 2:w * ROW2 + 66],
                                  in_=ps2[:, 0:64])
            nc.vector.tensor_copy(out=ad2win[:, w:w + 1], in_=ps2[:, 65:66])
        nc.sync.dma_start(out=shard2[:].rearrange("(w p) r -> p w r", p=128),
                          in_=row2_all[:].rearrange("p (w r) -> p w r", r=ROW2))
        nc.sync.dma_start(out=shard2[PAD2:PAD2 + 1, 0:2], in_=T["pr2"][0:1, :])

    nc.gpsimd.collective_compute(
        "AllGather", OP.bypass,
        replica_groups=[list(range(NCORES))],
        ins=[shard2[:]], outs=[table2[:]],
    )

    with tc.tile_pool(name="acc2", bufs=1) as acc2p:
        den2 = acc2p.tile([128, NW], F32, tag="den2")
        opre2 = acc2p.tile([128, NW * OUT], F32, tag="opre2")
        edge_layer(2, den2, opre2)

        # ------------- epilogue 2: normalize, bias, log_softmax -------
        _epilogue2(nc, tc, T, den2, opre2)


def _epilogue2(nc, tc, T, den2, opre2):
    with tc.tile_pool(name="epi2", bufs=1) as epip:
        nc.vector.tensor_scalar_max(den2[:], den2[:], 1e-30)
        rec2 = epip.tile([128, NW], F32, tag="rec2")
        nc.vector.reciprocal(rec2[:], den2[:])
        o2 = opre2
        nc.vector.tensor_tensor(
            out=o2[:].rearrange("p (w c) -> p w c", c=OUT),
            in0=o2[:].rearrange("p (w c) -> p w c", c=OUT),
            in1=rec2[:].rearrange("p (w c) -> p w c", c=1)
                .to_broadcast([128, NW, OUT]),
            op=OP.mult)
        nc.vector.tensor_tensor(
            out=o2[:].rearrange("p (w x) -> p w x", x=OUT),
            in0=o2[:].rearrange("p (w x) -> p w x", x=OUT),
            in1=T["b2_t"][:].rearrange("p (a x) -> p a x", a=1)
                .to_broadcast([128, NW, OUT]),
            op=OP.add)
        mx = epip.tile([128, NW], F32, tag="mx")
        nc.vector.tensor_reduce(
            out=mx[:], in_=o2[:].rearrange("p (w c) -> p w c", c=OUT),
            axis=AX.X, op=OP.max)
        nc.vector.tensor_tensor(
            out=o2[:].rearrange("p (w c) -> p w c", c=OUT),
            in0=o2[:].rearrange("p (w c) -> p w c", c=OUT),
            in1=mx[:].rearrange("p (w c) -> p w c", c=1)
                .to_broadcast([128, NW, OUT]),
            op=OP.subtract)
        ex = epip.tile([128, NW * OUT], F32, tag="ex")
        nc.scalar.activation(ex[:], o2[:], ACT.Exp, 0.0, 1.0)
        se = epip.tile([128, NW], F32, tag="se")
        nc.vector.tensor_reduce(
            out=se[:], in_=ex[:].rearrange("p (w c) -> p w c", c=OUT),
            axis=AX.X, op=OP.add)
        ln = epip.tile([128, NW], F32, tag="ln")
        nc.scalar.activation(ln[:], se[:], ACT.Ln, 0.0, 1.0)
        nc.vector.tensor_tensor(
            out=ex[:].rearrange("p (w c) -> p w c", c=OUT),
            in0=o2[:].rearrange("p (w c) -> p w c", c=OUT),
            in1=ln[:].rearrange("p (w c) -> p w c", c=1)
                .to_broadcast([128, NW, OUT]),
            op=OP.subtract)
        nc.sync.dma_start(out=T["outp"][:].rearrange("(w p) f -> p w f", p=128),
                          in_=ex[:].rearrange("p (w f) -> p w f", f=OUT))


_CACHE = {}


def kernel(**inputs):
    ei = np.asarray(inputs["edge_index"])
    src, dst = ei[0].astype(np.int64), ei[1].astype(np.int64)
    lay = _layout(src, dst)
    batches = _make_batches(lay["Lg"])
    per_core = _host_inputs(inputs, lay, batches)
    key = (ei.tobytes()[:64], tuple(tuple(lmg) for _, lmg in batches))
    if key not in _CACHE:
        _CACHE[key] = _build_program(batches)
    nc = _CACHE[key]
    res = run_bass_kernel_spmd(nc, per_core, core_ids=list(range(NCORES)))
    out = np.empty((N, OUT), np.float32)
    for k in range(NCORES):
        out[k * NSH + lay["perms"][k]] = res.results[k]["out"][:NSH]
    return out


if __name__ == "__main__":
    d = np.load("/root/problem/_inp_check.npz")
    o = kernel(**{k: d[k] for k in d.files})
    ref = np.load("/root/problem/_ref_check.npy")
    rel = np.linalg.norm(o - ref) / np.linalg.norm(ref)
    err = np.abs(o - ref) / (np.abs(ref) + 1e-5)
    print("fro rel err:", rel, "max elem rel err:", err.max())


# revision 29
# speedup vs baseline: 1.4495x; 1.0630x over previous
"""GAT (2-layer) Trainium2 Bass kernel — 8-core SPMD, pipelined gathers.

Sharding: dst nodes across 8 cores (12500 each). Per core, dsts are packed
into 98 windows of 128 (one SBUF partition per dst), profile-sorted (lexsort
on per-src-group degree vectors) so slot padding is small. Edges are gathered
per window as rows [a_s f32 | h bf16] from per-layer node tables
(4 src-groups, int16 indices) via tile-managed gpsimd.dma_gather — no
critical sections, 4 SWDGE queues and a 3-deep staging pool, so gathers
pipeline with DVE compute automatically. Slot layout per batch is
[group][window][Lmax_g] (per-group uniform Lmax), which makes every
attention op a single strided DVE instruction and segment-sums a
two-stage tensor_reduce.
Pad slots point at a row with a_s=-300 => weights ~e^-56, no masking needed.
Epilogues (ELU + layer-2 projection, log_softmax) run batched over all
windows at layer end; layer-2 node table is built per-shard and AllGathered.
"""

import numpy as np
import ml_dtypes

import concourse.bacc as bacc
import concourse.bass as bass
import concourse.mybir as mybir
import concourse.tile as tile
from concourse.bass_utils import run_bass_kernel_spmd
from concourse.masks import make_identity

F32 = mybir.dt.float32
BF16 = mybir.dt.bfloat16
I16 = mybir.dt.int16
AX = mybir.AxisListType
OP = mybir.AluOpType
ACT = mybir.ActivationFunctionType

N, E = 100000, 1600000
IN, HID, OUT, HEADS = 256, 16, 64, 8
NEG = 0.2
NCORES = 8
NSH = N // NCORES        # 12500
NGRP = 4
GSZ = N // NGRP          # 25000
NP = 25088               # padded rows per group (196*128)
NW = 98                  # windows per core
SH_ROWS = NW * 128       # 12544
PAD1 = GSZ               # group-local pad row, table1 (25000; rows 25000.. zero-x)
PAD2 = NSH               # pair-local pad row, table2 (row 12500 of even shard)
MAXC_BUDGET = 88         # slot columns per gather batch
WPB = 1                  # windows per batch
ROW1 = 256               # bf16 elems per table1 row (512B)
ROW2 = 128               # bf16 elems per table2 row (256B)
A_S_NEG = -300.0
TPB = 196                # 128-node tiles per src group
CHUNK = 14               # tiles per phase-0 staging chunk


# ---------------------------------------------------------------- host side
def _layout(src, dst):
    core = dst // NSH
    grp = src // GSZ
    cg_all = np.zeros((NCORES, NSH, NGRP), np.int32)
    np.add.at(cg_all, (core, dst % NSH, grp), 1)
    perms = []
    for k in range(NCORES):
        cg = cg_all[k]
        perms.append(np.lexsort((cg[:, 3], cg[:, 2], cg[:, 1], cg[:, 0]))[::-1])
    Lg = np.zeros((NW, NGRP), np.int64)
    for k in range(NCORES):
        cgp = cg_all[k][perms[k]]
        cgp = np.concatenate([cgp, np.zeros((SH_ROWS - NSH, NGRP), np.int32)])
        Lg = np.maximum(Lg, cgp.reshape(NW, 128, NGRP).max(axis=1))
    Lmaxw = Lg.max(axis=1)           # per-window uniform slot count
    sig = np.empty(N, np.int64)
    for k in range(NCORES):
        pos = np.empty(NSH, np.int64)
        pos[perms[k]] = np.arange(NSH)
        sig[k * NSH:(k + 1) * NSH] = k * SH_ROWS + pos
    eorder = np.lexsort((grp, dst))
    es, ed, eg, ec = src[eorder], dst[eorder], grp[eorder], core[eorder]
    core_starts = np.searchsorted(ec, np.arange(NCORES + 1))
    cores = [(es[a:b], (ed[a:b] - k * NSH), eg[a:b])
             for k, (a, b) in enumerate(zip(core_starts[:-1], core_starts[1:]))]
    return dict(Lg=Lg, Lmaxw=Lmaxw, perms=perms, sig=sig, cores=cores)


def _make_batches(Lg):
    """Batches of <=WPB windows; per-group uniform Lmax within a batch.
    Returns [(ws, lmg)] with lmg = per-group slot count (len NGRP)."""
    batches = []
    w = 0
    while w < NW:
        take = 1
        if (w + 1 < NW) and WPB >= 2:
            lmg = Lg[w:w + 2].max(axis=0)
            if 2 * int(lmg.sum()) <= MAXC_BUDGET:
                take = 2
        ws = list(range(w, w + take))
        lmg = [max(1, int(v)) for v in Lg[ws].max(axis=0)]
        batches.append((ws, lmg))
        w += take
    return batches


def _pack_idx(arr_pj):
    """[128, cols] slot-array of indices -> wrapped idx tile [128, cols*8]."""
    I = arr_pj.T.ravel()                      # I[j*128+p]
    W = I.reshape(-1, 16).T.astype(np.int16)  # [16, len/16]
    return np.tile(W, (8, 1))


def _host_inputs(inputs, lay, batches):
    x = np.asarray(inputs["x"], np.float32)
    W1 = np.asarray(inputs["W1"], np.float64)
    att1_s = np.asarray(inputs["att1_s"], np.float64)
    att1_d = np.asarray(inputs["att1_d"], np.float64)
    W2 = np.asarray(inputs["W2"], np.float64)
    att2_s = np.asarray(inputs["att2_s"], np.float64)
    att2_d = np.asarray(inputs["att2_d"], np.float64)
    b1 = np.asarray(inputs["b1"], np.float32)
    b2 = np.asarray(inputs["b2"], np.float32)
    Lg, perms, sig = lay["Lg"], lay["perms"], lay["sig"]

    A_s = np.zeros((HEADS * HID, HEADS))
    A_d = np.zeros((HEADS * HID, HEADS))
    for h in range(HEADS):
        A_s[h * HID:(h + 1) * HID, h] = att1_s[h]
        A_d[h * HID:(h + 1) * HID, h] = att1_d[h]
    w1r = np.concatenate([W1, W1 @ A_s, W1 @ A_d], axis=1)          # [256,144]
    w2r = np.concatenate([W2, W2 @ att2_s.T, W2 @ att2_d.T], axis=1)  # [128,66]
    w1r_bf = w1r.astype(ml_dtypes.bfloat16)
    w2r_bf = w2r.astype(ml_dtypes.bfloat16)

    xT = np.zeros((IN, NGRP * NP), np.float32)
    for g in range(NGRP):
        xT[:, g * NP:g * NP + GSZ] = x[g * GSZ:(g + 1) * GSZ].T
    xT_bf = xT.astype(ml_dtypes.bfloat16)

    common = {
        "xt0": np.ascontiguousarray(xT_bf[:128]),
        "xt1": np.ascontiguousarray(xT_bf[128:]),
        "w1r0": np.ascontiguousarray(w1r_bf[:128]),
        "w1r1": np.ascontiguousarray(w1r_bf[128:]),
        "w2r": np.ascontiguousarray(w2r_bf),
        "b1rep": np.ascontiguousarray(np.tile(b1[None, :], (128, 1)).astype(np.float32)),
        "b2rep": np.ascontiguousarray(np.tile(b2[None, :], (128, 1)).astype(np.float32)),
    }

    per_core = []
    for k in range(NCORES):
        es, edl, eg = lay["cores"][k]
        pos = np.empty(NSH, np.int64)
        pos[perms[k]] = np.arange(NSH)
        o = np.lexsort((eg, pos[edl]))
        es_o, eg_o, pos_o = es[o], eg[o], pos[edl][o]
        w_o, p_o = pos_o // 128, pos_o % 128
        key = pos_o * NGRP + eg_o
        slot = np.arange(len(o)) - np.searchsorted(key, key)
        idx1_secs, idx2_secs = [], []
        for ws, lmg in batches:
            Wn = len(ws)
            for g in range(NGRP):
                lm = lmg[g]
                a1 = np.full((128, Wn * lm), PAD1, np.int64)
                a2 = np.full((128, Wn * lm), PAD2, np.int64)
                for wl, w in enumerate(ws):
                    m = (w_o == w) & (eg_o == g)
                    pp, jj, ss = p_o[m], slot[m], es_o[m]
                    a1[pp, wl * lm + jj] = ss % GSZ
                    a2[pp, wl * lm + jj] = sig[ss] % NP
                idx1_secs.append(a1)
                idx2_secs.append(a2)
        idx1 = np.concatenate([_pack_idx(a) for a in idx1_secs], axis=1)
        idx2 = np.concatenate([_pack_idx(a) for a in idx2_secs], axis=1)
        xtp = np.zeros((IN, SH_ROWS), np.float32)
        xtp[:, :NSH] = x[k * NSH:(k + 1) * NSH].T[:, perms[k]]
        xtp_bf = xtp.astype(ml_dtypes.bfloat16)
        d = dict(common)
        d["idx1"] = np.ascontiguousarray(idx1)
        d["idx2"] = np.ascontiguousarray(idx2)
        d["xtp0"] = np.ascontiguousarray(xtp_bf[:128])
        d["xtp1"] = np.ascontiguousarray(xtp_bf[128:])
        per_core.append(d)
    return per_core


# ------------------------------------------------------------- device side
GATHER_VARIANT = "full"   # "full" | "tiny" (timing experiment: 128-row gathers)


def _build_program(batches, nrepeat=1):
    nc = bacc.Bacc("TRN2", target_bir_lowering=False, debug=False,
                   num_devices=NCORES, num_swdge_queues=4)
    MAXC = max(len(ws) * sum(lmg) for ws, lmg in batches)
    IDXF = sum(len(ws) * sum(lmg) * 8 for ws, lmg in batches)
    xt0 = nc.declare_dram_parameter("xt0", [128, NGRP * NP], BF16, isOutput=False)
    xt1 = nc.declare_dram_parameter("xt1", [128, NGRP * NP], BF16, isOutput=False)
    w1r0 = nc.declare_dram_parameter("w1r0", [128, 144], BF16, isOutput=False)
    w1r1 = nc.declare_dram_parameter("w1r1", [128, 144], BF16, isOutput=False)
    w2rp = nc.declare_dram_parameter("w2r", [128, 66], BF16, isOutput=False)
    b1rep = nc.declare_dram_parameter("b1rep", [128, 128], F32, isOutput=False)
    b2rep = nc.declare_dram_parameter("b2rep", [128, 64], F32, isOutput=False)
    idx1 = nc.declare_dram_parameter("idx1", [128, IDXF], I16, isOutput=False)
    idx2 = nc.declare_dram_parameter("idx2", [128, IDXF], I16, isOutput=False)
    xtp0 = nc.declare_dram_parameter("xtp0", [128, SH_ROWS], BF16, isOutput=False)
    xtp1 = nc.declare_dram_parameter("xtp1", [128, SH_ROWS], BF16, isOutput=False)
    outp = nc.declare_dram_parameter("out", [SH_ROWS, OUT], F32, isOutput=True)

    table1 = nc.dram_tensor("table1", [NGRP * NP, ROW1], BF16)
    shard2 = nc.dram_tensor("shard2", [SH_ROWS, ROW2], BF16)
    table2 = nc.dram_tensor("table2", [NCORES * SH_ROWS, ROW2], BF16,
                            addr_space="Shared")

    with tile.TileContext(nc) as tc:
        with tc.tile_pool(name="const", bufs=1) as constp:
            w1r0_t = constp.tile([128, 144], BF16, tag="w1r0")
            w1r1_t = constp.tile([128, 144], BF16, tag="w1r1")
            w2r_t = constp.tile([128, 66], BF16, tag="w2r")
            b1_t = constp.tile([128, 128], F32, tag="b1")
            b2_t = constp.tile([128, 64], F32, tag="b2")
            ident = constp.tile([128, 128], BF16, tag="ident")
            adwin = constp.tile([128, NW * HEADS], F32, tag="adwin")
            ad2win = constp.tile([128, NW], F32, tag="ad2win")
            padrow = constp.tile([128, ROW1], BF16, tag="padrow")
            pr2 = constp.tile([1, 2], BF16, tag="pr2")
            nc.sync.dma_start(out=w1r0_t[:], in_=w1r0[:])
            nc.sync.dma_start(out=w1r1_t[:], in_=w1r1[:])
            nc.sync.dma_start(out=w2r_t[:], in_=w2rp[:])
            nc.sync.dma_start(out=b1_t[:], in_=b1rep[:])
            nc.sync.dma_start(out=b2_t[:], in_=b2rep[:])
            make_identity(nc, ident[:])
            nc.vector.memset(padrow[:], 0.0)
            nc.vector.memset(padrow[0:1, 0:16].bitcast(F32), A_S_NEG)
            nc.vector.memset(pr2[0:1, 0:2].bitcast(F32), A_S_NEG)

            for _rep in range(nrepeat):
                _emit_body(nc, tc, batches, MAXC,
                           dict(xt0=xt0, xt1=xt1, idx1=idx1, idx2=idx2,
                                xtp0=xtp0, xtp1=xtp1, outp=outp,
                                table1=table1, shard2=shard2, table2=table2,
                                w1r0_t=w1r0_t, w1r1_t=w1r1_t, w2r_t=w2r_t,
                                b1_t=b1_t, b2_t=b2_t, ident=ident,
                                adwin=adwin, ad2win=ad2win, padrow=padrow,
                                pr2=pr2))
    nc.compile()
    return nc


def _emit_body(nc, tc, batches, MAXC, T):
    xt0, xt1 = T["xt0"], T["xt1"]
    table1, shard2, table2 = T["table1"], T["shard2"], T["table2"]
    w1r0_t, w1r1_t, w2r_t = T["w1r0_t"], T["w1r1_t"], T["w2r_t"]
    adwin, ad2win = T["adwin"], T["ad2win"]

    # ---------------- phase 0: dense h1 table (all nodes) ----------
    PSG = 3  # psum group: 3 node-tiles per PSUM bank
    with (
        tc.tile_pool(name="xt", bufs=2) as xtpool,
        tc.tile_pool(name="psum0", bufs=4, space="PSUM") as psump,
        tc.tile_pool(name="rows", bufs=2) as rowsp,
    ):
        for g in range(NGRP):
            for blk in range(TPB // CHUNK):
                base = g * NP + blk * CHUNK * 128
                xs0 = xtpool.tile([128, CHUNK * 128], BF16, tag="xs0")
                xs1 = xtpool.tile([128, CHUNK * 128], BF16, tag="xs1")
                nc.sync.dma_start(out=xs0[:], in_=xt0[:, base:base + CHUNK * 128])
                nc.sync.dma_start(out=xs1[:], in_=xt1[:, base:base + CHUNK * 128])
                rows = rowsp.tile([128, CHUNK * ROW1], BF16, tag="rows")
                t = 0
                while t < CHUNK:
                    npg = min(PSG, CHUNK - t)
                    ps = psump.tile([128, PSG * 144], F32, tag="ps0")
                    for u in range(npg):
                        tt = t + u
                        nc.tensor.matmul(
                            out=ps[:, u * 144:(u + 1) * 144],
                            lhsT=xs0[:, tt * 128:(tt + 1) * 128],
                            rhs=w1r0_t[:], start=True, stop=False)
                        nc.tensor.matmul(
                            out=ps[:, u * 144:(u + 1) * 144],
                            lhsT=xs1[:, tt * 128:(tt + 1) * 128],
                            rhs=w1r1_t[:], start=False, stop=True)
                    # a_s: rows[:, t*256 + {0..16}] (strided over npg tiles)
                    rv = rows[:, t * ROW1:(t + npg) * ROW1]
                    nc.vector.tensor_copy(
                        out=rv.rearrange("p (a r) -> p a r", a=npg)[:, :, 0:16]
                            .bitcast(F32),
                        in_=ps[:, 0:npg * 144].rearrange("p (a r) -> p a r", a=npg)
                            [:, :, 128:136])
                    nc.vector.tensor_copy(
                        out=rv.rearrange("p (a r) -> p a r", a=npg)[:, :, 16:144],
                        in_=ps[:, 0:npg * 144].rearrange("p (a r) -> p a r", a=npg)
                            [:, :, 0:128])
                    t += npg
                nc.sync.dma_start(
                    out=table1[base:base + CHUNK * 128, :]
                        .rearrange("(a p) r -> p a r", p=128),
                    in_=rows[:].rearrange("p (a r) -> p a r", a=CHUNK))
        # pad row: a_s := -300 (h stays 0) on group-local row PAD1
        for g in range(NGRP):
            nc.sync.dma_start(out=table1[g * NP + PAD1:g * NP + PAD1 + 1, :],
                              in_=T["padrow"][0:1, :])

        # a_d per window (window-ordered x.T): 14 windows per chunk
        for blk in range(NW // CHUNK):
            base = blk * CHUNK * 128
            xp0 = xtpool.tile([128, CHUNK * 128], BF16, tag="xp0")
            xp1 = xtpool.tile([128, CHUNK * 128], BF16, tag="xp1")
            nc.sync.dma_start(out=xp0[:], in_=T["xtp0"][:, base:base + CHUNK * 128])
            nc.sync.dma_start(out=xp1[:], in_=T["xtp1"][:, base:base + CHUNK * 128])
            half = CHUNK // 2  # 7 windows per psum tile
            for hb in range(2):
                psa = psump.tile([128, 7 * 16], F32, tag="psa")
                for u in range(half):
                    wloc = hb * half + u
                    nc.tensor.matmul(out=psa[:, u * 16:(u + 1) * 16],
                                     lhsT=xp0[:, wloc * 128:(wloc + 1) * 128],
                                     rhs=w1r0_t[:, 128:144], start=True, stop=False)
                    nc.tensor.matmul(out=psa[:, u * 16:(u + 1) * 16],
                                     lhsT=xp1[:, wloc * 128:(wloc + 1) * 128],
                                     rhs=w1r1_t[:, 128:144], start=False, stop=True)
                w0 = blk * CHUNK + hb * half
                nc.vector.tensor_copy(
                    out=adwin[:, w0 * 8:(w0 + half) * 8]
                        .rearrange("p (a h) -> p a h", a=half),
                    in_=psa[:].rearrange("p (a r) -> p a r", a=half)[:, :, 8:16])

    # ---------------- edge layers ----------------------------------
    def edge_layer(layer, den_t, opre_t):
        tabl, row_e = (table1, ROW1) if layer == 1 else (table2, ROW2)
        idxin = T["idx1"] if layer == 1 else T["idx2"]
        nh = HEADS if layer == 1 else 1
        nch = HID if layer == 1 else OUT
        hcw = nh * nch                      # 128 / 64
        hoff = 16 if layer == 1 else 2      # bf16 slots before h
        adv_t = adwin if layer == 1 else ad2win
        idx_off = 0
        qload = [0, 0, 0, 0]   # balance gathered rows across SWDGE rings

        with (
            tc.tile_pool(name=f"stag{layer}", bufs=3) as stagp,
            tc.tile_pool(name=f"idx{layer}", bufs=2) as idxp,
            tc.tile_pool(name=f"wall{layer}", bufs=2) as wallp,
        ):
            for ws, lmg in batches:
                Wn = len(ws)
                assert Wn == 1, "edge_layer assumes single-window batches"
                lsum = sum(lmg)
                cols = Wn * lsum
                goff = [Wn * sum(lmg[:g]) for g in range(NGRP + 1)]
                ixt = idxp.tile([128, MAXC * 8], I16, tag="ix")
                nc.sync.dma_start(
                    out=ixt[:, 0:cols * 8],
                    in_=idxin[:, idx_off:idx_off + cols * 8])
                idx_off += cols * 8
                stag = stagp.tile([128, MAXC * ROW1], BF16, tag="st")
                for g in sorted(range(NGRP), key=lambda g: -lmg[g]):
                    nidx_g = 128 * Wn * lmg[g]
                    ncols_g = Wn * lmg[g]
                    if GATHER_VARIANT == "tiny":
                        nidx_g, ncols_g = 128, 1
                    q = min(range(4), key=lambda i: qload[i])
                    qload[q] += nidx_g
                    sl3 = stag[:, goff[g] * row_e:(goff[g] + ncols_g) * row_e] \
                        .rearrange("p (k d) -> p k d", d=row_e)
                    nc.gpsimd.dma_gather(
                        out_ap=sl3, in_ap=tabl[g * NP:(g + 1) * NP, :],
                        idxs_ap=ixt[:, goff[g] * 8:goff[g] * 8 + nidx_g // 16],
                        num_idxs=nidx_g, num_idxs_reg=nidx_g,
                        elem_size=row_e, single_packet=False,
                        queue_num=q)
                # single window per batch: the 4 group sections are
                # contiguous and share one a_d, so every op is whole-batch
                w = ws[0]
                sec = stag[:, 0:cols * row_e]
                wall_t = wallp.tile([128, MAXC * HEADS], F32, tag="wa")
                lr_t = wallp.tile([128, MAXC * HEADS], F32, tag="lr")
                wall = wall_t[:, 0:cols * nh]
                a_s = sec.rearrange("p (l d) -> p l d", d=row_e) \
                    [:, :, 0:2 * nh].bitcast(F32)
                adv = adv_t[:, w * nh:(w + 1) * nh] \
                    .rearrange("p (a h) -> p a h", a=1) \
                    .to_broadcast([128, cols, nh])
                nc.vector.tensor_tensor(
                    out=wall.rearrange("p (l h) -> p l h", h=nh),
                    in0=a_s, in1=adv, op=OP.add)
                # leaky-relu + exp on the whole batch
                nc.vector.tensor_scalar_mul(lr_t[:, 0:cols * nh], wall, NEG)
                nc.vector.tensor_tensor(out=wall, in0=wall,
                                        in1=lr_t[:, 0:cols * nh], op=OP.max)
                nc.scalar.activation(wall, wall, ACT.Exp, 0.0, 1.0)
                # weighted messages, in place over the gathered h
                hview = sec.rearrange("p (x d) -> p x d", d=row_e) \
                    [:, :, hoff:hoff + hcw] \
                    .rearrange("p x (h c) -> p x h c", h=nh)
                wb = wall.rearrange("p (x h c) -> p x h c", h=nh, c=1) \
                    .to_broadcast([128, cols, nh, nch])
                nc.vector.tensor_tensor(out=hview, in0=hview, in1=wb, op=OP.mult)
                # one-stage segment reduction over all slots of the window
                nc.vector.tensor_reduce(
                    out=opre_t[:, w * hcw:(w + 1) * hcw],
                    in_=sec.rearrange("p (x d) -> p d x", d=row_e)
                        [:, hoff:hoff + hcw, :],
                    axis=AX.X, op=OP.add)
                nc.vector.tensor_reduce(
                    out=den_t[:, w * nh:(w + 1) * nh],
                    in_=wall.rearrange("p (x h) -> p h x", h=nh),
                    axis=AX.X, op=OP.add)

    with tc.tile_pool(name="acc1", bufs=1) as acc1p:
        den_all = acc1p.tile([128, NW * HEADS], F32, tag="den1")
        opre_all = acc1p.tile([128, NW * 128], F32, tag="opre1")
        edge_layer(1, den_all, opre_all)

        # ------------- epilogue 1: normalize, ELU, project, shard2 ----
        with (
            tc.tile_pool(name="epi1", bufs=1) as epip,
            tc.tile_pool(name="sm1", bufs=4) as smallp,
            tc.tile_pool(name="psum1", bufs=4, space="PSUM") as psump,
        ):
        nc.vector.tensor_scalar_max(den_all[:], den_all[:], 1e-30)
        rec = epip.tile([128, NW * HEADS], F32, tag="rec")
        nc.vector.reciprocal(rec[:], den_all[:])
        o1 = opre_all
        nc.vector.tensor_tensor(
            out=o1[:].rearrange("p (v c) -> p v c", c=HID),
            in0=o1[:].rearrange("p (v c) -> p v c", c=HID),
            in1=rec[:].rearrange("p (v c) -> p v c", c=1)
                .to_broadcast([128, NW * HEADS, HID]),
            op=OP.mult)
        nc.vector.tensor_tensor(
            out=o1[:].rearrange("p (w x) -> p w x", x=128),
            in0=o1[:].rearrange("p (w x) -> p w x", x=128),
            in1=T["b1_t"][:].rearrange("p (a x) -> p a x", a=1)
                .to_broadcast([128, NW, 128]),
            op=OP.add)
        tneg = epip.tile([128, NW * 128], F32, tag="tneg")
        nc.vector.tensor_scalar_min(tneg[:], o1[:], 0.0)
        nc.scalar.activation(tneg[:], tneg[:], ACT.Exp, 0.0, 1.0)
        nc.vector.tensor_relu(o1[:], o1[:])
        nc.vector.tensor_tensor(out=o1[:], in0=o1[:], in1=tneg[:], op=OP.add)
        nc.vector.tensor_scalar_add(o1[:], o1[:], -1.0)
        o1bf = epip.tile([128, NW * 128], BF16, tag="o1bf")
        nc.vector.tensor_copy(out=o1bf[:], in_=o1[:])
        row2_all = epip.tile([128, NW * ROW2], BF16, tag="row2")
        nc.vector.memset(row2_all[:], 0.0)
        for w in range(NW):
            pst = psump.tile([128, 128], BF16, tag="pst")
            nc.tensor.transpose(out=pst[:], in_=o1bf[:, w * 128:(w + 1) * 128],
                                identity=T["ident"][:])
            o1T = smallp.tile([128, 128], BF16, tag="o1T")
            nc.vector.tensor_copy(out=o1T[:], in_=pst[:])
            ps2 = psump.tile([128, 66], F32, tag="ps2")
            nc.tensor.matmul(out=ps2[:], lhsT=o1T[:], rhs=w2r_t[:],
                             start=True, stop=True)
            nc.vector.tensor_copy(
                out=row2_all[:, w * ROW2:w * ROW2 + 2].bitcast(F32),
                in_=ps2[:, 64:65])
            nc.vector.tensor_copy(out=row2_all[:, w * ROW2 + 2:w * ROW2 + 66],
                                  in_=ps2[:, 0:64])
            nc.vector.tensor_copy(out=ad2win[:, w:w + 1], in_=ps2[:, 65:66])
        nc.sync.dma_start(out=shard2[:].rearrange("(w p) r -> p w r", p=128),
                          in_=row2_all[:].rearrange("p (w r) -> p w r", r=ROW2))
        nc.sync.dma_start(out=shard2[PAD2:PAD2 + 1, 0:2], in_=T["pr2"][0:1, :])

    nc.gpsimd.collective_compute(
        "AllGather", OP.bypass,
        replica_groups=[list(range(NCORES))],
        ins=[shard2[:]], outs=[table2[:]],
    )

    with tc.tile_pool(name="acc2", bufs=1) as acc2p:
        den2 = acc2p.tile([128, NW], F32, tag="den2")
        opre2 = acc2p.tile([128, NW * OUT], F32, tag="opre2")
        edge_layer(2, den2, opre2)

        # ------------- epilogue 2: normalize, bias, log_softmax -------
        _epilogue2(nc, tc, T, den2, opre2)


def _epilogue2(nc, tc, T, den2, opre2):
    with tc.tile_pool(name="epi2", bufs=1) as epip:
        nc.vector.tensor_scalar_max(den2[:], den2[:], 1e-30)
        rec2 = epip.tile([128, NW], F32, tag="rec2")
        nc.vector.reciprocal(rec2[:], den2[:])
        o2 = opre2
        nc.vector.tensor_tensor(
            out=o2[:].rearrange("p (w c) -> p w c", c=OUT),
            in0=o2[:].rearrange("p (w c) -> p w c", c=OUT),
            in1=rec2[:].rearrange("p (w c) -> p w c", c=1)
                .to_broadcast([128, NW, OUT]),
            op=OP.mult)
        nc.vector.tensor_tensor(
            out=o2[:].rearrange("p (w x) -> p w x", x=OUT),
            in0=o2[:].rearrange("p (w x) -> p w x", x=OUT),
            in1=T["b2_t"][:].rearrange("p (a x) -> p a x", a=1)
                .to_broadcast([128, NW, OUT]),
            op=OP.add)
        mx = epip.tile([128, NW], F32, tag="mx")
        nc.vector.tensor_reduce(
            out=mx[:], in_=o2[:].rearrange("p (w c) -> p w c", c=OUT),
            axis=AX.X, op=OP.max)
        nc.vector.tensor_tensor(
            out=o2[:].rearrange("p (w c) -> p w c", c=OUT),
            in0=o2[:].rearrange("p (w c) -> p w c", c=OUT),
            in1=mx[:].rearrange("p (w c) -> p w c", c=1)
                .to_broadcast([128, NW, OUT]),
            op=OP.subtract)
        ex = epip.tile([128, NW * OUT], F32, tag="ex")
        nc.scalar.activation(ex[:], o2[:], ACT.Exp, 0.0, 1.0)
        se = epip.tile([128, NW], F32, tag="se")
        nc.vector.tensor_reduce(
            out=se[:], in_=ex[:].rearrange("p (w c) -> p w c", c=OUT),
            axis=AX.X, op=OP.add)
        ln = epip.tile([128, NW], F32, tag="ln")
        nc.scalar.activation(ln[:], se[:], ACT.Ln, 0.0, 1.0)
        nc.vector.tensor_tensor(
            out=ex[:].rearrange("p (w c) -> p w c", c=OUT),
            in0=o2[:].rearrange("p (w c) -> p w c", c=OUT),
            in1=ln[:].rearrange("p (w c) -> p w c", c=1)
                .to_broadcast([128, NW, OUT]),
            op=OP.subtract)
        nc.sync.dma_start(out=T["outp"][:].rearrange("(w p) f -> p w f", p=128),
                          in_=ex[:].rearrange("p (w f) -> p w f", f=OUT))


_CACHE = {}


def kernel(**inputs):
    ei = np.asarray(inputs["edge_index"])
    src, dst = ei[0].astype(np.int64), ei[1].astype(np.int64)
    lay = _layout(src, dst)
    batches = _make_batches(lay["Lg"])
    per_core = _host_inputs(inputs, lay, batches)
    key = (ei.tobytes()[:64], tuple(tuple(lmg) for _, lmg in batches))
    if key not in _CACHE:
        _CACHE[key] = _build_program(batches)
    nc = _CACHE[key]
    res = run_bass_kernel_spmd(nc, per_core, core_ids=list(range(NCORES)))
    out = np.empty((N, OUT), np.float32)
    for k in range(NCORES):
        out[k * NSH + lay["perms"][k]] = res.results[k]["out"][:NSH]
    return out


if __name__ == "__main__":
    d = np.load("/root/problem/_inp_check.npz")
    o = kernel(**{k: d[k] for k in d.files})
    ref = np.load("/root/problem/_ref_check.npy")
    rel = np.linalg.norm(o - ref) / np.linalg.norm(ref)
    err = np.abs(o - ref) / (np.abs(ref) + 1e-5)
    print("fro rel err:", rel, "max elem rel err:", err.max())
